# revision 26
# baseline (speedup 1.0000x reference)
"""Trainium2 Bass kernel for nn_DenseAttention (sparse_attention, C=31, B=D=1024).

Strategy (class-parallel over 8 NeuronCores):
- Each core handles 4 classes (core 7: 3 real + 1 zero dummy).
- Single-term fp16 path (validated: end-to-end scale-rel absmax err ~6.5e-3
  vs the 2e-2 gate): xBT = K_c^T fp16 matmul of fp16 x with fp32 PSUM
  accumulate, bias added during the PSUM->SBUF fp16 copy; xBBx logits on the
  upper cross-domain half block, label-equality masking (multiply), then
  E = exp(logits - 200) shipped to host as fp32.
- The reference's softmax is a raw reshape [B,B,C] -> [C, B*B]: softmax groups
  are 31 chunks of 2^20 flat elements crossing class boundaries. Host computes
  the exact per-element group id g = (31*(i*B+j) + c) >> 20 and accumulates
  group sums in fp64 (bincount), then out = sum_c E_c / s_g.
- exp shift is the constant 200 (any per-group-constant shift cancels in the
  softmax ratio; 200 keeps everything in fp32 range and reproduces the
  reference's masked-element underflow-to-zero behaviour exactly).
- Class schedule is software-pipelined (m1 of class cl+1 issued before m2 of
  class cl, double-buffered xB) so the tensor queue never waits on the
  PSUM->SBUF copies between the two matmuls.
"""

import functools

import numpy as np

import concourse.mybir as mybir
import concourse.tile as tile
from concourse import bacc
from concourse.bass_utils import run_bass_kernel_spmd

C, B, D = 31, 1024, 1024
NCORES = 8
CPAD = 4
MHAT = 200.0
F32 = mybir.dt.float32
F16 = mybir.dt.float16
EXP = mybir.ActivationFunctionType.Exp
IDENT = mybir.ActivationFunctionType.Identity
ALU = mybir.AluOpType


@functools.lru_cache(maxsize=1)
def _build():
    nc = bacc.Bacc("TRN2", target_bir_lowering=False, debug=False,
                   num_devices=NCORES)
    xth_d = nc.dram_tensor("xth", [8, 128, 1024], F16, kind="ExternalInput")
    # class 0 weights in dc-major layout: khi0[dc][p, et*128+e] = K0[dc*128+p,
    # et*128+e]; classes 1..3 in et-major layout as before
    khi0_d = nc.dram_tensor("khi0", [8, 128, 1024], F16, kind="ExternalInput")
    khi_d = nc.dram_tensor("khi", [CPAD - 1, 8, 128, 1024], F16,
                           kind="ExternalInput")
    labi_d = nc.dram_tensor("labi", [128, CPAD * 8], F32, kind="ExternalInput")
    labj_d = nc.dram_tensor("labj", [128, CPAD * 512], F16,
                            kind="ExternalInput")
    bias_d = nc.dram_tensor("biasc", [128, CPAD * 8], F32, kind="ExternalInput")
    # E upper cross block per (class, i-tile); host mirrors the lower block
    oute_d = nc.dram_tensor("out_e", [128, CPAD * 4 * 512], F32,
                            kind="ExternalOutput")

    with tile.TileContext(nc) as tc:
        with (
            tc.tile_pool(name="persist", bufs=1) as pp,
            tc.tile_pool(name="kpool", bufs=4) as kp,
            tc.tile_pool(name="work", bufs=3) as wp,
            tc.tile_pool(name="psum", bufs=2, space="PSUM") as ps,
        ):
            xth_t = pp.tile([128, 8 * 1024], F16)
            xbh = [pp.tile([128, 8 * 1024], F16, name="xbh_a"),
                   pp.tile([128, 8 * 1024], F16, name="xbh_b")]
            labi_t = pp.tile([128, CPAD * 8], F32)
            labj_t = pp.tile([128, CPAD * 512], F16)
            bias_t = pp.tile([128, CPAD * 8], F32)
            b200_t = pp.tile([128, 1], F32)

            kh0_t = pp.tile([128, 8 * 1024], F16)

            # first matmul's operands lead both queues so real PE work starts
            # as early as possible
            nc.sync.dma_start(out=xth_t[:, 0:512], in_=xth_d[0][:, 0:512])
            for ih in range(2):
                for dc in range(8):
                    if ih == 0 and dc == 0:
                        continue
                    csl = slice(dc * 1024 + ih * 512, dc * 1024 + ih * 512 + 512)
                    nc.sync.dma_start(out=xth_t[:, csl],
                                      in_=xth_d[dc][:, ih * 512:ih * 512 + 512])
                    if ih == 0 and dc == 3:
                        nc.sync.dma_start(out=bias_t[:], in_=bias_d[:])
            nc.vector.memset(b200_t[:], -MHAT)

            def emit_m1_first():
                # class 0, dc-major: per ih pass, 8 PSUM banks accumulate all
                # et tiles while xth/kh0 stream in chunk-by-chunk
                xb = xbh[0]
                nc.gpsimd.dma_start(out=kh0_t[:, 0:128], in_=khi0_d[0][:, 0:128])
                nc.gpsimd.dma_start(out=kh0_t[:, 128:512],
                                    in_=khi0_d[0][:, 128:512])
                nc.gpsimd.dma_start(out=kh0_t[:, 512:1024],
                                    in_=khi0_d[0][:, 512:1024])
                for dc in range(1, 8):
                    nc.gpsimd.dma_start(out=kh0_t[:, dc * 1024:(dc + 1) * 1024],
                                        in_=khi0_d[dc])
                tags = ["p1", "p1", "p2", "p2", "q1", "q1", "q2", "q2"]
                for ih in range(2):
                    pts = [ps.tile([128, 512], F32, tag=tags[k],
                                   name=f"mm0_{ih}_{k}") for k in range(8)]
                    for dc in range(8):
                        for et in range(8):
                            nc.tensor.matmul(
                                out=pts[et][:],
                                lhsT=kh0_t[:, dc * 1024 + et * 128:
                                           dc * 1024 + et * 128 + 128],
                                rhs=xth_t[:, dc * 1024 + ih * 512:
                                          dc * 1024 + ih * 512 + 512],
                                start=(dc == 0), stop=(dc == 7))
                    for et in range(8):
                        osl = slice(et * 1024 + ih * 512,
                                    et * 1024 + ih * 512 + 512)
                        bsl = bias_t[:, et:et + 1]
                        if et % 2 == 0:
                            nc.scalar.activation(out=xb[:, osl], in_=pts[et][:],
                                                 func=IDENT, bias=bsl, scale=1.0)
                        else:
                            nc.vector.tensor_scalar(out=xb[:, osl],
                                                    in0=pts[et][:], scalar1=bsl,
                                                    scalar2=None, op0=ALU.add)

            def emit_m1(cl):
                # xBT[e, i] = sum_d K[d,e] * xT[d,i] (+bias), fp16 out
                xb = xbh[cl % 2]
                for et in range(8):
                    kh_t = kp.tile([128, 1024], F16, tag="kh")
                    nc.gpsimd.dma_start(out=kh_t[:], in_=khi_d[cl - 1, et])
                    pa = ps.tile([128, 512], F32, tag="p1")
                    pb = ps.tile([128, 512], F32, tag="p2")
                    pt = [pa, pb]
                    for dc in range(8):
                        w = kh_t[:, dc * 128:(dc + 1) * 128]
                        for ih in range(2):
                            nc.tensor.matmul(
                                out=pt[ih][:], lhsT=w,
                                rhs=xth_t[:, dc * 1024 + ih * 512:
                                          dc * 1024 + ih * 512 + 512],
                                start=(dc == 0), stop=(dc == 7))
                    bsl = bias_t[:, cl * 8 + et:cl * 8 + et + 1]
                    for ih in range(2):
                        osl = slice(et * 1024 + ih * 512,
                                    et * 1024 + ih * 512 + 512)
                        if ih == 0:
                            nc.scalar.activation(out=xb[:, osl], in_=pt[ih][:],
                                                 func=IDENT, bias=bsl, scale=1.0)
                        else:
                            nc.vector.tensor_scalar(out=xb[:, osl],
                                                    in0=pt[ih][:], scalar1=bsl,
                                                    scalar2=None, op0=ALU.add)
                if cl == 1:
                    # m2-only inputs: issued here so the startup HBM burst
                    # holds only what the first matmuls need
                    nc.sync.dma_start(out=labi_t[:], in_=labi_d[:])
                    nc.sync.dma_start(out=labj_t[:], in_=labj_d[:])

            def emit_m2(cl):
                # logits[i, j] on the upper cross block, mask, exp, ship out
                xb = xbh[cl % 2]
                for it in range(4):
                    # split the final tile so its mask/exp/DMA tail overlaps
                    # the second half's matmuls
                    halves = 2 if (cl == CPAD - 1 and it == 3) else 1
                    jw = 512 // halves
                    for jh in range(halves):
                        q1 = ps.tile([128, 512], F32, tag="q1",
                                     name=f"q1_{cl}_{it}_{jh}")
                        for ec in range(8):
                            ioff = ec * 1024 + it * 128
                            joff = ec * 1024 + 512 + jh * jw
                            nc.tensor.matmul(
                                out=q1[:, 0:jw], lhsT=xb[:, ioff:ioff + 128],
                                rhs=xb[:, joff:joff + jw],
                                start=(ec == 0), stop=(ec == 7))
                        mt = wp.tile([128, 512], F32, tag="mt",
                                     name=f"mt_{cl}_{it}_{jh}")
                        ext = wp.tile([128, 512], F32, tag="ext",
                                      name=f"ext_{cl}_{it}_{jh}")
                        jsl = slice(cl * 512 + jh * jw, cl * 512 + jh * jw + jw)
                        # mt = (labi == labj) * q1 in one pass
                        nc.vector.scalar_tensor_tensor(
                            out=mt[:, 0:jw], in0=labj_t[:, jsl],
                            scalar=labi_t[:, cl * 8 + it:cl * 8 + it + 1],
                            in1=q1[:, 0:jw], op0=ALU.is_equal, op1=ALU.mult)
                        nc.scalar.activation(out=ext[:, 0:jw], in_=mt[:, 0:jw],
                                             func=EXP, bias=b200_t[:], scale=1.0)
                        eoff = (cl * 4 + it) * 512 + jh * jw
                        nc.sync.dma_start(out=oute_d[:, eoff:eoff + jw],
                                          in_=ext[:, 0:jw])

            emit_m1_first()
            for cl in range(CPAD):
                if cl + 1 < CPAD:
                    emit_m1(cl + 1)
                emit_m2(cl)

    nc.compile()
    return nc


def _core_classes():
    return [list(range(c * 4, min(c * 4 + 4, C))) for c in range(NCORES)]


def _prep_inputs(x, labels, kernel, bias):
    xT = np.ascontiguousarray(x.T).astype(np.float16)
    xth = np.ascontiguousarray(xT.reshape(8, 128, 1024))
    in_maps = []
    for classes in _core_classes():
        k4 = np.zeros((CPAD, D, D), np.float32)
        b4 = np.zeros((CPAD, D), np.float32)
        l4 = np.zeros((B, CPAD), np.int32)
        for cl, c in enumerate(classes):
            k4[cl] = kernel[c]
            b4[cl] = bias[c]
            l4[:, cl] = labels[:, c]
        k16 = k4.astype(np.float16)
        # class 0 dc-major: [d, e] -> [dc(8), p(128), et(8)*128+e]
        khi0 = np.ascontiguousarray(k16[0].reshape(8, 128, 1024))
        # classes 1..3 et-major: [cl, d, e] -> [cl, et(8), p(128), dc(8)*128+e]
        khi = k16[1:].reshape(CPAD - 1, 8, 128, 8, 128)   # cl, dc, p, et, e
        khi = np.ascontiguousarray(khi.transpose(0, 3, 2, 1, 4))  # cl,et,p,dc,e
        khi = khi.reshape(CPAD - 1, 8, 128, 1024)
        labi = l4.reshape(8, 128, CPAD).transpose(1, 2, 0)      # p, cl, it
        labi = np.ascontiguousarray(labi.astype(np.float32)).reshape(
            128, CPAD * 8)
        labj = np.broadcast_to(
            l4[512:, :].T.astype(np.float16)[None, :, :], (128, CPAD, 512)
        ).reshape(128, CPAD * 512).copy()
        biasc = b4.reshape(CPAD, 8, 128).transpose(2, 0, 1)     # p, cl, et
        biasc = np.ascontiguousarray(biasc.astype(np.float32)).reshape(
            128, CPAD * 8)
        in_maps.append(dict(xth=xth, khi0=khi0, khi=khi, labi=labi, labj=labj,
                            biasc=biasc))
    return in_maps


def _assemble(results):
    # Full E per class (upper cross block shipped; E is symmetric)
    Efull = np.zeros((C, B, B), np.float32)
    for res, classes in zip(results, _core_classes()):
        eb = res["out_e"].reshape(128, CPAD, 4, 512)
        for cl, c in enumerate(classes):
            up = eb[:, cl].transpose(1, 0, 2).reshape(512, 512)
            Efull[c, :512, 512:] = up
            Efull[c, 512:, :512] = up.T
    # Exact flat-group softmax sums in fp64: element (i,j,c) lives at flat
    # position (i*B+j)*C + c; group = flat >> 20.
    idx = np.arange(B * B, dtype=np.int64)
    s = np.zeros(C, np.float64)
    for c in range(C):
        g = (idx * C + c) >> 20
        s += np.bincount(g, weights=Efull[c].reshape(-1).astype(np.float64),
                         minlength=C)
    out = np.zeros(B * B, np.float64)
    for c in range(C):
        g = (idx * C + c) >> 20
        out += Efull[c].reshape(-1).astype(np.float64) / s[g]
    return out.reshape(B, B).astype(np.float32)


def _run(inputs, trace=False):
    x = np.asarray(inputs["inputs"], np.float32)
    labels = np.asarray(inputs["labels"])
    kern = np.asarray(inputs["kernel"], np.float32)
    bias = np.asarray(inputs["bias"], np.float32)
    nc = _build()
    in_maps = _prep_inputs(x, labels, kern, bias)
    res = run_bass_kernel_spmd(nc, in_maps, core_ids=list(range(NCORES)),
                               trace=trace)
    out = _assemble(res.results)
    return out, res


def kernel(**inputs) -> np.ndarray:
    return _run(inputs, trace=False)[0]


# revision 27
# speedup vs baseline: 1.0092x; 1.0092x over previous
"""Trainium2 Bass kernel for nn_DenseAttention (sparse_attention, C=31, B=D=1024).

Strategy (class-parallel over 8 NeuronCores):
- Each core handles 4 classes (core 7: 3 real + 1 zero dummy).
- Single-term fp16 path (validated: end-to-end scale-rel absmax err ~6.5e-3
  vs the 2e-2 gate): xBT = K_c^T fp16 matmul of fp16 x with fp32 PSUM
  accumulate, bias added during the PSUM->SBUF fp16 copy; xBBx logits on the
  upper cross-domain half block, label-equality masking (multiply), then
  E = exp(logits - 200) shipped to host as fp32.
- The reference's softmax is a raw reshape [B,B,C] -> [C, B*B]: softmax groups
  are 31 chunks of 2^20 flat elements crossing class boundaries. Host computes
  the exact per-element group id g = (31*(i*B+j) + c) >> 20 and accumulates
  group sums in fp64 (bincount), then out = sum_c E_c / s_g.
- exp shift is the constant 200 (any per-group-constant shift cancels in the
  softmax ratio; 200 keeps everything in fp32 range and reproduces the
  reference's masked-element underflow-to-zero behaviour exactly).
- Class schedule is software-pipelined (m1 of class cl+1 issued before m2 of
  class cl, double-buffered xB) so the tensor queue never waits on the
  PSUM->SBUF copies between the two matmuls.
"""

import functools

import numpy as np

import concourse.mybir as mybir
import concourse.tile as tile
from concourse import bacc
from concourse.bass_utils import run_bass_kernel_spmd

C, B, D = 31, 1024, 1024
NCORES = 8
CPAD = 4
MHAT = 200.0
F32 = mybir.dt.float32
F16 = mybir.dt.float16
EXP = mybir.ActivationFunctionType.Exp
IDENT = mybir.ActivationFunctionType.Identity
ALU = mybir.AluOpType


@functools.lru_cache(maxsize=1)
def _build():
    nc = bacc.Bacc("TRN2", target_bir_lowering=False, debug=False,
                   num_devices=NCORES)
    xth_d = nc.dram_tensor("xth", [8, 128, 1024], F16, kind="ExternalInput")
    # class 0 weights in dc-major layout: khi0[dc][p, et*128+e] = K0[dc*128+p,
    # et*128+e]; classes 1..3 in et-major layout as before
    khi0_d = nc.dram_tensor("khi0", [8, 128, 1024], F16, kind="ExternalInput")
    khi_d = nc.dram_tensor("khi", [CPAD - 1, 8, 128, 1024], F16,
                           kind="ExternalInput")
    labi_d = nc.dram_tensor("labi", [128, CPAD * 8], F32, kind="ExternalInput")
    labj_d = nc.dram_tensor("labj", [128, CPAD * 512], F16,
                            kind="ExternalInput")
    bias_d = nc.dram_tensor("biasc", [128, CPAD * 8], F32, kind="ExternalInput")
    # E upper cross block per (class, i-tile); host mirrors the lower block
    oute_d = nc.dram_tensor("out_e", [128, CPAD * 4 * 512], F32,
                            kind="ExternalOutput")

    with tile.TileContext(nc) as tc:
        with (
            tc.tile_pool(name="persist", bufs=1) as pp,
            tc.tile_pool(name="kpool", bufs=4) as kp,
            tc.tile_pool(name="work", bufs=3) as wp,
            tc.tile_pool(name="psum", bufs=2, space="PSUM") as ps,
        ):
            xth_t = pp.tile([128, 8 * 1024], F16)
            xbh = [pp.tile([128, 8 * 1024], F16, name="xbh_a"),
                   pp.tile([128, 8 * 1024], F16, name="xbh_b")]
            labi_t = pp.tile([128, CPAD * 8], F32)
            labj_t = pp.tile([128, CPAD * 512], F16)
            bias_t = pp.tile([128, CPAD * 8], F32)
            b200_t = pp.tile([128, 1], F32)

            kh0_t = pp.tile([128, 8 * 1024], F16)

            # first matmul's operands lead both queues so real PE work starts
            # as early as possible
            nc.sync.dma_start(out=xth_t[:, 0:512], in_=xth_d[0][:, 0:512])
            for ih in range(2):
                for dc in range(8):
                    if ih == 0 and dc == 0:
                        continue
                    csl = slice(dc * 1024 + ih * 512, dc * 1024 + ih * 512 + 512)
                    nc.sync.dma_start(out=xth_t[:, csl],
                                      in_=xth_d[dc][:, ih * 512:ih * 512 + 512])
                    if ih == 0 and dc == 3:
                        nc.sync.dma_start(out=bias_t[:], in_=bias_d[:])
            nc.vector.memset(b200_t[:], -MHAT)

            def emit_m1_first():
                # class 0, dc-major: per ih pass, 8 PSUM banks accumulate all
                # et tiles while xth/kh0 stream in chunk-by-chunk
                xb = xbh[0]
                nc.gpsimd.dma_start(out=kh0_t[:, 0:128], in_=khi0_d[0][:, 0:128])
                nc.gpsimd.dma_start(out=kh0_t[:, 128:1024],
                                    in_=khi0_d[0][:, 128:1024])
                for dc in range(1, 8):
                    nc.gpsimd.dma_start(out=kh0_t[:, dc * 1024:(dc + 1) * 1024],
                                        in_=khi0_d[dc])
                tags = ["p1", "p1", "p2", "p2", "q1", "q1", "q2", "q2"]
                for ih in range(2):
                    pts = [ps.tile([128, 512], F32, tag=tags[k],
                                   name=f"mm0_{ih}_{k}") for k in range(8)]
                    for dc in range(8):
                        for et in range(8):
                            nc.tensor.matmul(
                                out=pts[et][:],
                                lhsT=kh0_t[:, dc * 1024 + et * 128:
                                           dc * 1024 + et * 128 + 128],
                                rhs=xth_t[:, dc * 1024 + ih * 512:
                                          dc * 1024 + ih * 512 + 512],
                                start=(dc == 0), stop=(dc == 7))
                    for et in range(8):
                        osl = slice(et * 1024 + ih * 512,
                                    et * 1024 + ih * 512 + 512)
                        bsl = bias_t[:, et:et + 1]
                        if et % 2 == 0:
                            nc.scalar.activation(out=xb[:, osl], in_=pts[et][:],
                                                 func=IDENT, bias=bsl, scale=1.0)
                        else:
                            nc.vector.tensor_scalar(out=xb[:, osl],
                                                    in0=pts[et][:], scalar1=bsl,
                                                    scalar2=None, op0=ALU.add)

            def emit_m1(cl):
                # xBT[e, i] = sum_d K[d,e] * xT[d,i] (+bias), fp16 out
                xb = xbh[cl % 2]
                for et in range(8):
                    kh_t = kp.tile([128, 1024], F16, tag="kh")
                    nc.gpsimd.dma_start(out=kh_t[:], in_=khi_d[cl - 1, et])
                    pa = ps.tile([128, 512], F32, tag="p1")
                    pb = ps.tile([128, 512], F32, tag="p2")
                    pt = [pa, pb]
                    for dc in range(8):
                        w = kh_t[:, dc * 128:(dc + 1) * 128]
                        for ih in range(2):
                            nc.tensor.matmul(
                                out=pt[ih][:], lhsT=w,
                                rhs=xth_t[:, dc * 1024 + ih * 512:
                                          dc * 1024 + ih * 512 + 512],
                                start=(dc == 0), stop=(dc == 7))
                    bsl = bias_t[:, cl * 8 + et:cl * 8 + et + 1]
                    for ih in range(2):
                        osl = slice(et * 1024 + ih * 512,
                                    et * 1024 + ih * 512 + 512)
                        if ih == 0:
                            nc.scalar.activation(out=xb[:, osl], in_=pt[ih][:],
                                                 func=IDENT, bias=bsl, scale=1.0)
                        else:
                            nc.vector.tensor_scalar(out=xb[:, osl],
                                                    in0=pt[ih][:], scalar1=bsl,
                                                    scalar2=None, op0=ALU.add)
                if cl == 1:
                    # m2-only inputs: issued here so the startup HBM burst
                    # holds only what the first matmuls need
                    nc.sync.dma_start(out=labi_t[:], in_=labi_d[:])
                    nc.sync.dma_start(out=labj_t[:], in_=labj_d[:])

            def emit_m2(cl):
                # logits[i, j] on the upper cross block, mask, exp, ship out
                xb = xbh[cl % 2]
                for it in range(4):
                    # split the final tile so its mask/exp/DMA tail overlaps
                    # the second half's matmuls
                    halves = 2 if (cl == CPAD - 1 and it == 3) else 1
                    jw = 512 // halves
                    for jh in range(halves):
                        q1 = ps.tile([128, 512], F32, tag="q1",
                                     name=f"q1_{cl}_{it}_{jh}")
                        for ec in range(8):
                            ioff = ec * 1024 + it * 128
                            joff = ec * 1024 + 512 + jh * jw
                            nc.tensor.matmul(
                                out=q1[:, 0:jw], lhsT=xb[:, ioff:ioff + 128],
                                rhs=xb[:, joff:joff + jw],
                                start=(ec == 0), stop=(ec == 7))
                        mt = wp.tile([128, 512], F32, tag="mt",
                                     name=f"mt_{cl}_{it}_{jh}")
                        ext = wp.tile([128, 512], F32, tag="ext",
                                      name=f"ext_{cl}_{it}_{jh}")
                        jsl = slice(cl * 512 + jh * jw, cl * 512 + jh * jw + jw)
                        # mt = (labi == labj) * q1 in one pass
                        nc.vector.scalar_tensor_tensor(
                            out=mt[:, 0:jw], in0=labj_t[:, jsl],
                            scalar=labi_t[:, cl * 8 + it:cl * 8 + it + 1],
                            in1=q1[:, 0:jw], op0=ALU.is_equal, op1=ALU.mult)
                        nc.scalar.activation(out=ext[:, 0:jw], in_=mt[:, 0:jw],
                                             func=EXP, bias=b200_t[:], scale=1.0)
                        eoff = (cl * 4 + it) * 512 + jh * jw
                        nc.sync.dma_start(out=oute_d[:, eoff:eoff + jw],
                                          in_=ext[:, 0:jw])

            emit_m1_first()
            for cl in range(CPAD):
                if cl + 1 < CPAD:
                    emit_m1(cl + 1)
                emit_m2(cl)

    nc.compile()
    return nc


def _core_classes():
    return [list(range(c * 4, min(c * 4 + 4, C))) for c in range(NCORES)]


def _prep_inputs(x, labels, kernel, bias):
    xT = np.ascontiguousarray(x.T).astype(np.float16)
    xth = np.ascontiguousarray(xT.reshape(8, 128, 1024))
    in_maps = []
    for classes in _core_classes():
        k4 = np.zeros((CPAD, D, D), np.float32)
        b4 = np.zeros((CPAD, D), np.float32)
        l4 = np.zeros((B, CPAD), np.int32)
        for cl, c in enumerate(classes):
            k4[cl] = kernel[c]
            b4[cl] = bias[c]
            l4[:, cl] = labels[:, c]
        k16 = k4.astype(np.float16)
        # class 0 dc-major: [d, e] -> [dc(8), p(128), et(8)*128+e]
        khi0 = np.ascontiguousarray(k16[0].reshape(8, 128, 1024))
        # classes 1..3 et-major: [cl, d, e] -> [cl, et(8), p(128), dc(8)*128+e]
        khi = k16[1:].reshape(CPAD - 1, 8, 128, 8, 128)   # cl, dc, p, et, e
        khi = np.ascontiguousarray(khi.transpose(0, 3, 2, 1, 4))  # cl,et,p,dc,e
        khi = khi.reshape(CPAD - 1, 8, 128, 1024)
        labi = l4.reshape(8, 128, CPAD).transpose(1, 2, 0)      # p, cl, it
        labi = np.ascontiguousarray(labi.astype(np.float32)).reshape(
            128, CPAD * 8)
        labj = np.broadcast_to(
            l4[512:, :].T.astype(np.float16)[None, :, :], (128, CPAD, 512)
        ).reshape(128, CPAD * 512).copy()
        biasc = b4.reshape(CPAD, 8, 128).transpose(2, 0, 1)     # p, cl, et
        biasc = np.ascontiguousarray(biasc.astype(np.float32)).reshape(
            128, CPAD * 8)
        in_maps.append(dict(xth=xth, khi0=khi0, khi=khi, labi=labi, labj=labj,
                            biasc=biasc))
    return in_maps


def _assemble(results):
    # Full E per class (upper cross block shipped; E is symmetric)
    Efull = np.zeros((C, B, B), np.float32)
    for res, classes in zip(results, _core_classes()):
        eb = res["out_e"].reshape(128, CPAD, 4, 512)
        for cl, c in enumerate(classes):
            up = eb[:, cl].transpose(1, 0, 2).reshape(512, 512)
            Efull[c, :512, 512:] = up
            Efull[c, 512:, :512] = up.T
    # Exact flat-group softmax sums in fp64: element (i,j,c) lives at flat
    # position (i*B+j)*C + c; group = flat >> 20.
    idx = np.arange(B * B, dtype=np.int64)
    s = np.zeros(C, np.float64)
    for c in range(C):
        g = (idx * C + c) >> 20
        s += np.bincount(g, weights=Efull[c].reshape(-1).astype(np.float64),
                         minlength=C)
    out = np.zeros(B * B, np.float64)
    for c in range(C):
        g = (idx * C + c) >> 20
        out += Efull[c].reshape(-1).astype(np.float64) / s[g]
    return out.reshape(B, B).astype(np.float32)


def _run(inputs, trace=False):
    x = np.asarray(inputs["inputs"], np.float32)
    labels = np.asarray(inputs["labels"])
    kern = np.asarray(inputs["kernel"], np.float32)
    bias = np.asarray(inputs["bias"], np.float32)
    nc = _build()
    in_maps = _prep_inputs(x, labels, kern, bias)
    res = run_bass_kernel_spmd(nc, in_maps, core_ids=list(range(NCORES)),
                               trace=trace)
    out = _assemble(res.results)
    return out, res


def kernel(**inputs) -> np.ndarray:
    return _run(inputs, trace=False)[0]


# revision 29
# speedup vs baseline: 1.0147x; 1.0055x over previous
"""Trainium2 Bass kernel for nn_DenseAttention (sparse_attention, C=31, B=D=1024).

Strategy (class-parallel over 8 NeuronCores):
- Each core handles 4 classes (core 7: 3 real + 1 zero dummy).
- Single-term fp16 path (validated: end-to-end scale-rel absmax err ~6.5e-3
  vs the 2e-2 gate): xBT = K_c^T fp16 matmul of fp16 x with fp32 PSUM
  accumulate, bias added during the PSUM->SBUF fp16 copy; xBBx logits on the
  upper cross-domain half block, label-equality masking (multiply), then
  E = exp(logits - 200) shipped to host as fp32.
- The reference's softmax is a raw reshape [B,B,C] -> [C, B*B]: softmax groups
  are 31 chunks of 2^20 flat elements crossing class boundaries. Host computes
  the exact per-element group id g = (31*(i*B+j) + c) >> 20 and accumulates
  group sums in fp64 (bincount), then out = sum_c E_c / s_g.
- exp shift is the constant 200 (any per-group-constant shift cancels in the
  softmax ratio; 200 keeps everything in fp32 range and reproduces the
  reference's masked-element underflow-to-zero behaviour exactly).
- Class schedule is software-pipelined (m1 of class cl+1 issued before m2 of
  class cl, double-buffered xB) so the tensor queue never waits on the
  PSUM->SBUF copies between the two matmuls.
"""

import functools

import numpy as np

import concourse.mybir as mybir
import concourse.tile as tile
from concourse import bacc
from concourse.bass_utils import run_bass_kernel_spmd

C, B, D = 31, 1024, 1024
NCORES = 8
CPAD = 4
MHAT = 200.0
F32 = mybir.dt.float32
F16 = mybir.dt.float16
EXP = mybir.ActivationFunctionType.Exp
IDENT = mybir.ActivationFunctionType.Identity
ALU = mybir.AluOpType


@functools.lru_cache(maxsize=1)
def _build():
    nc = bacc.Bacc("TRN2", target_bir_lowering=False, debug=False,
                   num_devices=NCORES)
    xth_d = nc.dram_tensor("xth", [8, 128, 1024], F16, kind="ExternalInput")
    # class 0 weights in dc-major layout: khi0[dc][p, et*128+e] = K0[dc*128+p,
    # et*128+e]; classes 1..3 in et-major layout as before
    khi0_d = nc.dram_tensor("khi0", [8, 128, 1024], F16, kind="ExternalInput")
    khi_d = nc.dram_tensor("khi", [CPAD - 1, 8, 128, 1024], F16,
                           kind="ExternalInput")
    labi_d = nc.dram_tensor("labi", [128, CPAD * 8], F32, kind="ExternalInput")
    labj_d = nc.dram_tensor("labj", [128, CPAD * 512], F16,
                            kind="ExternalInput")
    bias_d = nc.dram_tensor("biasc", [128, CPAD * 8], F32, kind="ExternalInput")
    # E upper cross block per (class, i-tile); host mirrors the lower block
    oute_d = nc.dram_tensor("out_e", [128, CPAD * 4 * 512], F32,
                            kind="ExternalOutput")

    with tile.TileContext(nc) as tc:
        with (
            tc.tile_pool(name="persist", bufs=1) as pp,
            tc.tile_pool(name="kpool", bufs=4) as kp,
            tc.tile_pool(name="work", bufs=3) as wp,
            tc.tile_pool(name="psum", bufs=2, space="PSUM") as ps,
        ):
            xth_t = pp.tile([128, 8 * 1024], F16)
            xbh = [pp.tile([128, 8 * 1024], F16, name="xbh_a"),
                   pp.tile([128, 8 * 1024], F16, name="xbh_b")]
            labi_t = pp.tile([128, CPAD * 8], F32)
            labj_t = pp.tile([128, CPAD * 512], F16)
            bias_t = pp.tile([128, CPAD * 8], F32)
            b200_t = pp.tile([128, 1], F32)

            kh0_t = pp.tile([128, 8 * 1024], F16)

            # first matmul's operands lead both queues so real PE work starts
            # as early as possible; dc=1 weights ride the sync queue so the
            # dc-major pass isn't serialized behind dc=0's weight transfers
            nc.sync.dma_start(out=xth_t[:, 0:512], in_=xth_d[0][:, 0:512])
            nc.sync.dma_start(out=kh0_t[:, 1024:2048], in_=khi0_d[1])
            for ih in range(2):
                for dc in range(8):
                    if ih == 0 and dc == 0:
                        continue
                    csl = slice(dc * 1024 + ih * 512, dc * 1024 + ih * 512 + 512)
                    nc.sync.dma_start(out=xth_t[:, csl],
                                      in_=xth_d[dc][:, ih * 512:ih * 512 + 512])
                    if ih == 0 and dc == 3:
                        nc.sync.dma_start(out=bias_t[:], in_=bias_d[:])
            nc.vector.memset(b200_t[:], -MHAT)

            def emit_m1_first():
                # class 0, dc-major: per ih pass, 8 PSUM banks accumulate all
                # et tiles while xth/kh0 stream in chunk-by-chunk
                xb = xbh[0]
                nc.gpsimd.dma_start(out=kh0_t[:, 0:128], in_=khi0_d[0][:, 0:128])
                nc.gpsimd.dma_start(out=kh0_t[:, 128:1024],
                                    in_=khi0_d[0][:, 128:1024])
                for dc in range(2, 8):
                    nc.gpsimd.dma_start(out=kh0_t[:, dc * 1024:(dc + 1) * 1024],
                                        in_=khi0_d[dc])
                tags = ["p1", "p1", "p2", "p2", "q1", "q1", "q2", "q2"]
                for ih in range(2):
                    pts = [ps.tile([128, 512], F32, tag=tags[k],
                                   name=f"mm0_{ih}_{k}") for k in range(8)]
                    for dc in range(8):
                        for et in range(8):
                            nc.tensor.matmul(
                                out=pts[et][:],
                                lhsT=kh0_t[:, dc * 1024 + et * 128:
                                           dc * 1024 + et * 128 + 128],
                                rhs=xth_t[:, dc * 1024 + ih * 512:
                                          dc * 1024 + ih * 512 + 512],
                                start=(dc == 0), stop=(dc == 7))
                    for et in range(8):
                        osl = slice(et * 1024 + ih * 512,
                                    et * 1024 + ih * 512 + 512)
                        bsl = bias_t[:, et:et + 1]
                        if et % 2 == 0:
                            nc.scalar.activation(out=xb[:, osl], in_=pts[et][:],
                                                 func=IDENT, bias=bsl, scale=1.0)
                        else:
                            nc.vector.tensor_scalar(out=xb[:, osl],
                                                    in0=pts[et][:], scalar1=bsl,
                                                    scalar2=None, op0=ALU.add)

            def emit_m1(cl):
                # xBT[e, i] = sum_d K[d,e] * xT[d,i] (+bias), fp16 out
                xb = xbh[cl % 2]
                for et in range(8):
                    kh_t = kp.tile([128, 1024], F16, tag="kh")
                    nc.gpsimd.dma_start(out=kh_t[:], in_=khi_d[cl - 1, et])
                    pa = ps.tile([128, 512], F32, tag="p1")
                    pb = ps.tile([128, 512], F32, tag="p2")
                    pt = [pa, pb]
                    for dc in range(8):
                        w = kh_t[:, dc * 128:(dc + 1) * 128]
                        for ih in range(2):
                            nc.tensor.matmul(
                                out=pt[ih][:], lhsT=w,
                                rhs=xth_t[:, dc * 1024 + ih * 512:
                                          dc * 1024 + ih * 512 + 512],
                                start=(dc == 0), stop=(dc == 7))
                    bsl = bias_t[:, cl * 8 + et:cl * 8 + et + 1]
                    for ih in range(2):
                        osl = slice(et * 1024 + ih * 512,
                                    et * 1024 + ih * 512 + 512)
                        if ih == 0:
                            nc.scalar.activation(out=xb[:, osl], in_=pt[ih][:],
                                                 func=IDENT, bias=bsl, scale=1.0)
                        else:
                            nc.vector.tensor_scalar(out=xb[:, osl],
                                                    in0=pt[ih][:], scalar1=bsl,
                                                    scalar2=None, op0=ALU.add)
                if cl == 1:
                    # m2-only inputs: issued here so the startup HBM burst
                    # holds only what the first matmuls need
                    nc.sync.dma_start(out=labi_t[:], in_=labi_d[:])
                    nc.sync.dma_start(out=labj_t[:], in_=labj_d[:])

            def emit_m2(cl):
                # logits[i, j] on the upper cross block, mask, exp, ship out
                xb = xbh[cl % 2]
                for it in range(4):
                    # split the final tile so its mask/exp/DMA tail overlaps
                    # the second half's matmuls
                    halves = 2 if (cl == CPAD - 1 and it == 3) else 1
                    jw = 512 // halves
                    for jh in range(halves):
                        q1 = ps.tile([128, 512], F32, tag="q1",
                                     name=f"q1_{cl}_{it}_{jh}")
                        for ec in range(8):
                            ioff = ec * 1024 + it * 128
                            joff = ec * 1024 + 512 + jh * jw
                            nc.tensor.matmul(
                                out=q1[:, 0:jw], lhsT=xb[:, ioff:ioff + 128],
                                rhs=xb[:, joff:joff + jw],
                                start=(ec == 0), stop=(ec == 7))
                        mt = wp.tile([128, 512], F32, tag="mt",
                                     name=f"mt_{cl}_{it}_{jh}")
                        ext = wp.tile([128, 512], F32, tag="ext",
                                      name=f"ext_{cl}_{it}_{jh}")
                        jsl = slice(cl * 512 + jh * jw, cl * 512 + jh * jw + jw)
                        # mt = (labi == labj) * q1 in one pass
                        nc.vector.scalar_tensor_tensor(
                            out=mt[:, 0:jw], in0=labj_t[:, jsl],
                            scalar=labi_t[:, cl * 8 + it:cl * 8 + it + 1],
                            in1=q1[:, 0:jw], op0=ALU.is_equal, op1=ALU.mult)
                        nc.scalar.activation(out=ext[:, 0:jw], in_=mt[:, 0:jw],
                                             func=EXP, bias=b200_t[:], scale=1.0)
                        eoff = (cl * 4 + it) * 512 + jh * jw
                        nc.sync.dma_start(out=oute_d[:, eoff:eoff + jw],
                                          in_=ext[:, 0:jw])

            emit_m1_first()
            for cl in range(CPAD):
                if cl + 1 < CPAD:
                    emit_m1(cl + 1)
                emit_m2(cl)

    nc.compile()
    return nc


def _core_classes():
    return [list(range(c * 4, min(c * 4 + 4, C))) for c in range(NCORES)]


def _prep_inputs(x, labels, kernel, bias):
    xT = np.ascontiguousarray(x.T).astype(np.float16)
    xth = np.ascontiguousarray(xT.reshape(8, 128, 1024))
    in_maps = []
    for classes in _core_classes():
        k4 = np.zeros((CPAD, D, D), np.float32)
        b4 = np.zeros((CPAD, D), np.float32)
        l4 = np.zeros((B, CPAD), np.int32)
        for cl, c in enumerate(classes):
            k4[cl] = kernel[c]
            b4[cl] = bias[c]
            l4[:, cl] = labels[:, c]
        k16 = k4.astype(np.float16)
        # class 0 dc-major: [d, e] -> [dc(8), p(128), et(8)*128+e]
        khi0 = np.ascontiguousarray(k16[0].reshape(8, 128, 1024))
        # classes 1..3 et-major: [cl, d, e] -> [cl, et(8), p(128), dc(8)*128+e]
        khi = k16[1:].reshape(CPAD - 1, 8, 128, 8, 128)   # cl, dc, p, et, e
        khi = np.ascontiguousarray(khi.transpose(0, 3, 2, 1, 4))  # cl,et,p,dc,e
        khi = khi.reshape(CPAD - 1, 8, 128, 1024)
        labi = l4.reshape(8, 128, CPAD).transpose(1, 2, 0)      # p, cl, it
        labi = np.ascontiguousarray(labi.astype(np.float32)).reshape(
            128, CPAD * 8)
        labj = np.broadcast_to(
            l4[512:, :].T.astype(np.float16)[None, :, :], (128, CPAD, 512)
        ).reshape(128, CPAD * 512).copy()
        biasc = b4.reshape(CPAD, 8, 128).transpose(2, 0, 1)     # p, cl, et
        biasc = np.ascontiguousarray(biasc.astype(np.float32)).reshape(
            128, CPAD * 8)
        in_maps.append(dict(xth=xth, khi0=khi0, khi=khi, labi=labi, labj=labj,
                            biasc=biasc))
    return in_maps


def _assemble(results):
    # Full E per class (upper cross block shipped; E is symmetric)
    Efull = np.zeros((C, B, B), np.float32)
    for res, classes in zip(results, _core_classes()):
        eb = res["out_e"].reshape(128, CPAD, 4, 512)
        for cl, c in enumerate(classes):
            up = eb[:, cl].transpose(1, 0, 2).reshape(512, 512)
            Efull[c, :512, 512:] = up
            Efull[c, 512:, :512] = up.T
    # Exact flat-group softmax sums in fp64: element (i,j,c) lives at flat
    # position (i*B+j)*C + c; group = flat >> 20.
    idx = np.arange(B * B, dtype=np.int64)
    s = np.zeros(C, np.float64)
    for c in range(C):
        g = (idx * C + c) >> 20
        s += np.bincount(g, weights=Efull[c].reshape(-1).astype(np.float64),
                         minlength=C)
    out = np.zeros(B * B, np.float64)
    for c in range(C):
        g = (idx * C + c) >> 20
        out += Efull[c].reshape(-1).astype(np.float64) / s[g]
    return out.reshape(B, B).astype(np.float32)


def _run(inputs, trace=False):
    x = np.asarray(inputs["inputs"], np.float32)
    labels = np.asarray(inputs["labels"])
    kern = np.asarray(inputs["kernel"], np.float32)
    bias = np.asarray(inputs["bias"], np.float32)
    nc = _build()
    in_maps = _prep_inputs(x, labels, kern, bias)
    res = run_bass_kernel_spmd(nc, in_maps, core_ids=list(range(NCORES)),
                               trace=trace)
    out = _assemble(res.results)
    return out, res


def kernel(**inputs) -> np.ndarray:
    return _run(inputs, trace=False)[0]
